# revision 8
# baseline (speedup 1.0000x reference)
"""HRqVae forward kernel for 8 Trainium2 NeuronCores (Bass/Tile).

Data-parallel over batch: each core handles 1024 of 8192 rows.
Codebooks + params replicated. The 8192x8192 InfoNCE logits are sharded
by query rows; normalized tag projections are exchanged via AllGather.
Scalar losses are finished on host from per-partition partial sums.
"""
import os
import sys
import numpy as np
from contextlib import ExitStack

sys.path.insert(0, "/opt/trn_rl_repo")

import concourse.bass as bass          # noqa: E402
import concourse.tile as tile          # noqa: E402
from concourse import bacc, mybir      # noqa: E402
from concourse.bass_utils import run_bass_kernel_spmd  # noqa: E402

dt = mybir.dt
AF = mybir.ActivationFunctionType
ALU = mybir.AluOpType
AX = mybir.AxisListType

B, IN, D, K, NL = 8192, 768, 128, 4096, 3
TAG = 768
COUNTS = [10, 100, 1000]
CW, ALIGN_W, TEMP = 0.25, 0.5, 0.1
NCORES = 8
BL = B // NCORES          # 1024 rows per core
NRT = BL // 128           # 8 row-tiles per core

F32, F32R = dt.float32, dt.float32r

USE_R = {
    "enc": os.environ.get("VQK_ENC", "32") == "r",
    "proj": os.environ.get("VQK_PROJ", "r") == "r",
    "nce": os.environ.get("VQK_NCE", "r") == "r",
    "pred": os.environ.get("VQK_PRED", "r") == "r",
    "dec": os.environ.get("VQK_DEC", "32") == "r",
}


def _chunks(d):
    return [(k * 128, min(128, d - k * 128)) for k in range((d + 127) // 128)]


def _wchunks(w):
    """[din, dout] -> [128, nk, dout] host layout for matmul rhs chunks."""
    din, dout = w.shape
    nk = (din + 127) // 128
    out = np.zeros((128, nk, dout), np.float32)
    for k, (o, n) in enumerate(_chunks(din)):
        out[:n, k, :] = w[o:o + n, :]
    return out


class Bld:
    def __init__(self, nc, tc, ctx):
        self.nc, self.tc, self.ctx = nc, tc, ctx
        self.pools = {}

    def tp(self, name, **kw):
        if name not in self.pools:
            self.pools[name] = self.ctx.enter_context(
                self.tc.tile_pool(name=name, **kw))
        return self.pools[name]

    def transpose(self, src_ap, n, dtype, pool, name="t", tag="chunkT"):
        nc = self.nc
        pt = self.tp("ps_t", bufs=2, space="PSUM").tile([128, 128], F32)
        nc.tensor.transpose(pt[:n, :], src_ap, self.ident[:])
        ch = pool.tile([128, 128], dtype, name="t", tag=tag)
        nc.vector.tensor_copy(ch[:n, :], pt[:n, :])
        return ch

    def actT(self, act, din, dtype, pool):
        return [(self.transpose(act[:, o:o + n], n, dtype, pool), n)
                for (o, n) in _chunks(din)]

    def matmul_psums(self, chT, w_tile, dout):
        nc = self.nc
        pp = self.tp("ps_m", bufs=4, space="PSUM")
        outs = []
        for (o, n) in [(c * 512, min(512, dout - c * 512))
                       for c in range((dout + 511) // 512)]:
            p = pp.tile([128, 512], F32)
            for k2, (ch, kn) in enumerate(chT):
                nc.tensor.matmul(p[:, :n], ch[:kn, :],
                                 w_tile[:kn, k2, o:o + n],
                                 start=(k2 == 0), stop=(k2 == len(chT) - 1))
            outs.append((p, n, o))
        return outs

    def linear(self, chT, w_tile, dout, func, out_pool, name="t", tag="act"):
        nc = self.nc
        out = out_pool.tile([128, dout], F32, name="t", tag=tag)
        for (p, n, o) in self.matmul_psums(chT, w_tile, dout):
            nc.scalar.activation(out[:, o:o + n], p[:, :n], func)
        return out

    def load_w(self, dram_ap, shape, rnd, pool, tag):
        """DMA weight chunks; optionally round to fp32r via staging."""
        nc = self.nc
        if not rnd:
            w = pool.tile(list(shape), F32, name="t", tag=tag)
            nc.sync.dma_start(w[:], dram_ap)
            return w
        st = self.tp("wstage", bufs=1).tile([128, 3072], F32, name="t", tag="st")
        fr = st[:, :shape[1] * shape[2]].rearrange(
            "p (k n) -> p k n", k=shape[1])
        nc.sync.dma_start(fr, dram_ap)
        w = pool.tile(list(shape), F32R, name="t", tag=tag)
        nc.vector.tensor_copy(w[:], fr)
        return w

    def layernorm(self, chT, w_tile, dout, relu, out_pool, name="t", tag="pa"):
        nc = self.nc
        sp = self.tp("small", bufs=16)
        psums = self.matmul_psums(chT, w_tile, dout)
        s1 = sp.tile([128, 1], F32, name="t", tag="s1")
        s2 = sp.tile([128, 1], F32, name="t", tag="s2")
        for i2, (p, n, o) in enumerate(psums):
            sq = self.tp("sq", bufs=2).tile([128, 512], F32, name="t", tag="sq")
            if i2 == 0:
                nc.vector.reduce_sum(s1[:], p[:, :n], axis=AX.X)
                nc.scalar.activation(sq[:, :n], p[:, :n], AF.Square,
                                     accum_out=s2[:])
            else:
                t1 = sp.tile([128, 1], F32, name="t", tag="t1")
                t2 = sp.tile([128, 1], F32, name="t", tag="t2")
                nc.vector.reduce_sum(t1[:], p[:, :n], axis=AX.X)
                nc.scalar.activation(sq[:, :n], p[:, :n], AF.Square,
                                     accum_out=t2[:])
                nc.vector.tensor_add(s1[:], s1[:], t1[:])
                nc.vector.tensor_add(s2[:], s2[:], t2[:])
        m = sp.tile([128, 1], F32, name="t", tag="m")
        nc.vector.tensor_scalar_mul(m[:], s1[:], 1.0 / dout)
        msq = sp.tile([128, 1], F32, name="t", tag="msq")
        nc.vector.tensor_tensor(out=msq[:], in0=m[:], in1=m[:], op=ALU.mult)
        var = sp.tile([128, 1], F32, name="t", tag="var")
        nc.vector.tensor_scalar(out=var[:], in0=s2[:], scalar1=1.0 / dout,
                                scalar2=msq[:], op0=ALU.mult, op1=ALU.subtract)
        std = sp.tile([128, 1], F32, name="t", tag="std")
        nc.scalar.activation(std[:], var[:], AF.Sqrt, bias=self.epsb[:])
        rstd = sp.tile([128, 1], F32, name="t", tag="rstd")
        nc.vector.reciprocal(rstd[:], std[:])
        nmr = sp.tile([128, 1], F32, name="t", tag="nmr")
        nc.vector.tensor_scalar(out=nmr[:], in0=m[:], scalar1=rstd[:],
                                scalar2=-1.0, op0=ALU.mult, op1=ALU.mult)
        out = out_pool.tile([128, dout], F32, name="t", tag=tag)
        fn = AF.Relu if relu else AF.Identity
        for (p, n, o) in psums:
            nc.scalar.activation(out[:, o:o + n], p[:, :n], fn,
                                 bias=nmr[:], scale=rstd[:])
        return out

    def l2n_scale(self, x, dout):
        nc = self.nc
        sp = self.tp("small", bufs=16)
        sq = self.tp("sq", bufs=2).tile([128, 512], F32, name="t", tag="sq")
        ss = sp.tile([128, 1], F32, name="t", tag="ss")
        nc.scalar.activation(sq[:, :min(dout, 512)], x[:, :min(dout, 512)],
                             AF.Square, accum_out=ss[:])
        sr = sp.tile([128, 1], F32, name="t", tag="sr")
        nc.scalar.activation(sr[:], ss[:], AF.Sqrt)
        rn = sp.tile([128, 1], F32, name="t", tag="rn")
        nc.vector.reciprocal(rn[:], sr[:])
        return rn


def build(n_counts=COUNTS):
    nc = bacc.Bacc("TRN2", target_bir_lowering=False, debug=False,
                   num_devices=NCORES)

    def inp(name, shape, dtype=F32):
        return nc.dram_tensor(name, list(shape), dtype,
                              kind="ExternalInput").ap()

    def outp(name, shape, dtype=F32):
        return nc.dram_tensor(name, list(shape), dtype,
                              kind="ExternalOutput").ap()

    xT_d = inp("xT", [IN, BL])
    tagsT_d = inp("tagsT", [NL, TAG, BL])
    lab_d = inp("lab", [BL, NL], dt.uint32)
    oh_d = [inp(f"oh{i}", [BL, n_counts[i]]) for i in range(NL)]
    cb_d = [inp(f"cb{i}_", [K, D]) for i in range(NL)]
    cbT2_d = inp("cbT2", [NL, 128, K])
    cb2b_d = inp("cb2b", [NL, 128, K])
    ident_d = inp("ident", [128, 128])
    ew_d = [inp(f"ew{i}", [128, kn, do]) for i, (kn, do) in
            enumerate([(6, 512), (4, 256), (2, 128), (1, 128)])]
    dw_d = [inp(f"dw{i}", [128, kn, do]) for i, (kn, do) in
            enumerate([(1, 128), (1, 256), (2, 512), (4, 768)])]
    p1_d = [inp(f"p1_{i}", [128, 6, 512]) for i in range(NL)]
    p2_d = [inp(f"p2_{i}", [128, 4, D * (i + 1)]) for i in range(NL)]
    PRED = ("a1", "a2", "fe", "r1a", "r1b", "r2a", "r2b", "ca", "cb")
    pw_d = {}
    for i in range(NL):
        Di, Hi, Mi, ncl = D * (i + 1), 256 * (i + 1), 256, n_counts[i]
        for nm, (a, b2) in dict(
                a1=(Di, Di // 4), a2=(Di // 4, Di), fe=(Di, Hi),
                r1a=(Hi, Mi), r1b=(Mi, Hi), r2a=(Hi, Mi), r2b=(Mi, Hi),
                ca=(Hi, Mi), cb=(Mi, ncl)).items():
            pw_d[(i, nm)] = (inp(f"pw{i}{nm}", [128, (a + 127) // 128, b2]),
                             [128, (a + 127) // 128, b2])

    embs_d = outp("embs", [NL, BL, D])
    resid_d = outp("resid", [NL, BL, D])
    ids_d = outp("ids", [NL, BL], dt.int32)
    xhat_d = outp("xhat", [BL, IN])
    lacc_d = outp("lacc", [128, 4])

    with tile.TileContext(nc) as tc, ExitStack() as ctx:
        bld = Bld(nc, tc, ctx)
        tp = bld.tp
        const = tp("const", bufs=1)
        bld.ident = const.tile([128, 128], F32, name="ident")
        nc.sync.dma_start(bld.ident[:], ident_d[:])
        bld.epsb = const.tile([128, 1], F32, name="epsb")
        nc.vector.memset(bld.epsb[:], 1e-5)
        sp = tp("small", bufs=16)
        accs = {}
        for nm in ("q", "a", "p", "pc"):
            accs[nm] = const.tile([128, 1], F32, name="t", tag=f"acc{nm}")
            nc.vector.memset(accs[nm][:], 0.0)

        cpool = tp("chunks", bufs=10)
        tp("ps_t", bufs=2, space="PSUM")
        tp("ps_m", bufs=4, space="PSUM")
        tp("sq", bufs=2)
        tp("sq2", bufs=1)
        tp("wstage", bufs=1)
        tp("res", bufs=12)
        tp("concat", bufs=8)
        tp("esum", bufs=8)
        ctpool = tp("concT", bufs=24)
        apool = tp("acts", bufs=3)
        ppool = tp("pacts", bufs=5)
        ppx = tp("pactx", bufs=2)
        dram = tp("dram", bufs=1, space="DRAM")
        RDT_NCE = F32R if USE_R["nce"] else F32

        # ---- phase 1: tag projector, tn AllGather (all levels) ----
        ag_out, tnb = [], []
        rdt_p = F32R if USE_R["proj"] else F32
        for i in range(NL):
            Di = D * (i + 1)
            with tc.tile_pool(name=f"projw{i}", bufs=1) as pwp:
                w1 = bld.load_w(p1_d[i][:], [128, 6, 512], USE_R["proj"],
                                pwp, "p1")
                w2 = bld.load_w(p2_d[i][:], [128, 4, Di], USE_R["proj"],
                                pwp, "p2")
                a_i = dram.tile([Di, BL], F32, name="t", tag=f"agin{i}")
                o_i = dram.tile([NCORES, Di, BL], F32, name="t", tag=f"agout{i}")
                ag_out.append(o_i)
                t_i = dram.tile([BL, Di], F32, name="t", tag=f"tnb{i}")
                tnb.append(t_i)
                for rt in range(NRT):
                    r0 = rt * 128
                    chT = []
                    for k2 in range(6):
                        cf = cpool.tile([128, 128], F32, name="t", tag="chunkF")
                        nc.sync.dma_start(
                            cf[:], tagsT_d[i, k2 * 128:(k2 + 1) * 128,
                                           r0:r0 + 128])
                        if USE_R["proj"]:
                            cr = cpool.tile([128, 128], F32R, name="t", tag="chunkT")
                            nc.vector.tensor_copy(cr[:], cf[:])
                            chT.append((cr, 128))
                        else:
                            chT.append((cf, 128))
                    h = bld.linear(chT, w1, 512, AF.Relu, apool, name="t", tag="h512")
                    hT = bld.actT(h, 512, rdt_p, cpool)
                    (p, n, _o) = bld.matmul_psums(hT, w2, Di)[0]
                    sq = tp("sq", bufs=2).tile([128, 512], F32, name="t", tag="sq")
                    ss = sp.tile([128, 1], F32, name="t", tag="ss")
                    nc.scalar.activation(sq[:, :n], p[:, :n], AF.Square,
                                         accum_out=ss[:])
                    sr = sp.tile([128, 1], F32, name="t", tag="sr")
                    nc.scalar.activation(sr[:], ss[:], AF.Sqrt)
                    rn = sp.tile([128, 1], F32, name="t", tag="rn")
                    nc.vector.reciprocal(rn[:], sr[:])
                    tn = apool.tile([128, Di], F32, name="t", tag="tn")
                    nc.scalar.activation(tn[:], p[:, :n], AF.Copy,
                                         scale=rn[:, 0:1])
                    nc.sync.dma_start(t_i[r0:r0 + 128, :], tn[:])
                    for (o, n2) in _chunks(Di):
                        tch = bld.transpose(tn[:, o:o + n2], n2, F32, cpool,
                                            name="t", tag="chunkF")
                        nc.sync.dma_start(a_i[o:o + n2, r0:r0 + 128],
                                          tch[:n2, :])
                nc.gpsimd.collective_compute(
                    "AllGather", ALU.bypass,
                    replica_groups=[list(range(NCORES))],
                    ins=[a_i[:]], outs=[o_i[:]])

        # ---- phase 2: encoder ----
        res_t = []
        respool = tp("res", bufs=12)
        with tc.tile_pool(name="encw", bufs=1) as mwp:
            enc_w = [bld.load_w(ew_d[li][:], list(ew_d[li].shape), False,
                                mwp, f"mw{li}") for li in range(4)]
            for rt in range(NRT):
                r0 = rt * 128
                chT = []
                for k2 in range(6):
                    cf = cpool.tile([128, 128], F32, name="t", tag="chunkF")
                    nc.sync.dma_start(cf[:], xT_d[k2 * 128:(k2 + 1) * 128,
                                                  r0:r0 + 128])
                    chT.append((cf, 128))
                h = bld.linear(chT, enc_w[0], 512, AF.Relu, apool, name="t", tag="h512")
                h = bld.linear(bld.actT(h, 512, F32, cpool), enc_w[1], 256,
                               AF.Relu, apool, name="t", tag="h512")
                h = bld.linear(bld.actT(h, 256, F32, cpool), enc_w[2], 128,
                               AF.Relu, apool, name="t", tag="h512")
                r = respool.tile([128, 128], F32, name="t", tag="res")
                (p, n, _o) = bld.matmul_psums(bld.actT(h, 128, F32, cpool),
                                              enc_w[3], 128)[0]
                nc.scalar.copy(r[:], p[:, :128])
                res_t.append(r)

        # ---- per level: VQ -> InfoNCE -> predictor ----
        concat = [tp("concat", bufs=8).tile([128, 384], F32, name="t", tag="cc")
                  for _ in range(NRT)]
        concT = [[None] * 3 for _ in range(NRT)]
        ssen = [None] * NRT
        esum = [tp("esum", bufs=8).tile([128, 128], F32, name="t", tag="es")
                for _ in range(NRT)]
        for i in range(NL):
            Di, ncl = D * (i + 1), n_counts[i]
            rn10 = []
            with tc.tile_pool(name=f"cbw{i}", bufs=1) as cbp, \
                    tc.tile_pool(name=f"sc{i}", bufs=1) as scp:
                cbw = cbp.tile([128, K], F32, name="t", tag="cbT2")
                nc.sync.dma_start(cbw[:], cbT2_d[i][:])
                c2b = cbp.tile([128, K], F32, name="t", tag="cb2b")
                nc.sync.dma_start(c2b[:], cb2b_d[i][:])
                for rt in range(NRT):
                    r0 = rt * 128
                    nc.sync.dma_start(resid_d[i, r0:r0 + 128, :], res_t[rt][:])
                    rT = bld.transpose(res_t[rt][:], 128, F32, cpool,
                                       name="t", tag="chunkF")
                    ssb = scp.tile([128, K], F32, name="t", tag="sc")
                    pp = tp("ps_m", bufs=4, space="PSUM")
                    for ct in range(K // 512):
                        o = ct * 512
                        p = pp.tile([128, 512], F32)
                        nc.tensor.matmul(p[:], rT[:], cbw[:, o:o + 512],
                                         start=True, stop=True)
                        nc.vector.tensor_tensor(out=ssb[:, o:o + 512],
                                                in0=p[:],
                                                in1=c2b[:, o:o + 512],
                                                op=ALU.subtract)
                    mx8 = sp.tile([128, 8], F32, name="t", tag="mx8")
                    mi8 = sp.tile([128, 8], dt.uint32, name="t", tag="mi8")
                    nc.vector.max(mx8[:], ssb[:])
                    nc.vector.max_index(mi8[:], mx8[:], ssb[:])
                    nc.sync.dma_start(ids_d[i, r0:r0 + 128].unsqueeze(1),
                                      mi8[:, 0:1].bitcast(dt.int32))
                    emb = concat[rt][:, i * 128:(i + 1) * 128]
                    nc.gpsimd.indirect_dma_start(
                        emb, None, cb_d[i][:],
                        bass.IndirectOffsetOnAxis(ap=mi8[:, 0:1], axis=0))
                    nc.sync.dma_start(embs_d[i, r0:r0 + 128, :], emb)
                    newr = respool.tile([128, 128], F32, name="t", tag="res")
                    nc.vector.tensor_tensor(out=newr[:], in0=res_t[rt][:],
                                            in1=emb, op=ALU.subtract)
                    sq = tp("sq", bufs=2).tile([128, 512], F32, name="t", tag="sq")
                    qp = sp.tile([128, 1], F32, name="t", tag="qp")
                    nc.scalar.activation(sq[:, :128], newr[:], AF.Square,
                                         accum_out=qp[:])
                    nc.vector.tensor_add(accs["q"][:], accs["q"][:], qp[:])
                    res_t[rt] = newr
                    if i == 0:
                        nc.vector.tensor_copy(esum[rt][:], emb)
                    else:
                        nc.vector.tensor_add(esum[rt][:], esum[rt][:], emb)
                    ep = sp.tile([128, 1], F32, name="t", tag="ep")
                    nc.scalar.activation(sq[:, :128], emb, AF.Square,
                                         accum_out=ep[:])
                    if i == 0:
                        ssen[rt] = const.tile([128, 1], F32, name="t", tag=f"ssen{rt}")
                        nc.vector.tensor_copy(ssen[rt][:], ep[:])
                    else:
                        nc.vector.tensor_add(ssen[rt][:], ssen[rt][:], ep[:])
                    concT[rt][i] = bld.transpose(emb, 128, RDT_NCE, ctpool,
                                                 name="t", tag="ct")
                    rr = sp.tile([128, 1], F32, name="t", tag="rr")
                    nc.scalar.activation(rr[:], ssen[rt][:], AF.Sqrt)
                    r10 = const.tile([128, 1], F32, name="t", tag=f"rn10_{rt}")
                    nc.vector.reciprocal(r10[:], rr[:])
                    nc.vector.tensor_scalar_mul(r10[:], r10[:], 1.0 / TEMP)
                    rn10.append(r10)
            # InfoNCE
            with tc.tile_pool(name=f"tnt{i}", bufs=4) as tntp, \
                    tc.tile_pool(name=f"sep{i}", bufs=8) as sepp:
                sep = [sepp.tile([128, 16], F32, name="t", tag="sep")
                       for _ in range(NRT)]
                for ct in range(B // 512):
                    cc, joff = ct // 2, (ct % 2) * 512
                    tch = []
                    for k2 in range(i + 1):
                        tf = tntp.tile([128, 512], F32, name="t", tag="tnf")
                        nc.sync.dma_start(
                            tf[:], ag_out[i][cc, k2 * 128:(k2 + 1) * 128,
                                             joff:joff + 512])
                        if USE_R["nce"]:
                            tr = tntp.tile([128, 512], F32R, name="t", tag="tnr")
                            nc.vector.tensor_copy(tr[:], tf[:])
                            tch.append(tr)
                        else:
                            tch.append(tf)
                    for rt in range(NRT):
                        pp = tp("ps_m", bufs=4, space="PSUM")
                        p = pp.tile([128, 512], F32)
                        for k2 in range(i + 1):
                            nc.tensor.matmul(p[:], concT[rt][k2][:],
                                             tch[k2][:], start=(k2 == 0),
                                             stop=(k2 == i))
                        esc = tp("sq", bufs=2).tile([128, 512], F32, name="t", tag="sq")
                        nc.scalar.activation(esc[:], p[:], AF.Exp,
                                             scale=rn10[rt][:, 0:1],
                                             accum_out=sep[rt][:, ct:ct + 1])
                for rt in range(NRT):
                    r0 = rt * 128
                    se = sp.tile([128, 1], F32, name="t", tag="se")
                    nc.vector.reduce_sum(se[:], sep[rt][:], axis=AX.X)
                    lse = sp.tile([128, 1], F32, name="t", tag="lse")
                    nc.scalar.activation(lse[:], se[:], AF.Ln)
                    tl_ = ppx.tile([128, Di], F32, name="t", tag="tnl")
                    nc.sync.dma_start(tl_[:], tnb[i][r0:r0 + 128, :])
                    dsc = ppx.tile([128, Di], F32, name="t", tag="dsc")
                    nc.vector.tensor_tensor(out=dsc[:],
                                            in0=concat[rt][:, :Di],
                                            in1=tl_[:], op=ALU.mult)
                    dg = sp.tile([128, 1], F32, name="t", tag="dg")
                    nc.vector.reduce_sum(dg[:], dsc[:], axis=AX.X)
                    per = sp.tile([128, 1], F32, name="t", tag="per")
                    nc.vector.tensor_scalar(out=per[:], in0=dg[:],
                                            scalar1=rn10[rt][:, 0:1],
                                            scalar2=lse[:], op0=ALU.mult,
                                            op1=ALU.subtract)
                    nc.vector.tensor_scalar_mul(per[:], per[:], -1.0)
                    nc.vector.tensor_add(accs["a"][:], accs["a"][:], per[:])
            # predictor
            Hi, Mi = 256 * (i + 1), 256
            rdt = F32R if USE_R["pred"] else F32
            with tc.tile_pool(name=f"predw{i}", bufs=1) as pwp:
                pw = {nm: bld.load_w(pw_d[(i, nm)][0][:], pw_d[(i, nm)][1],
                                     USE_R["pred"], pwp, f"pw{nm}")
                      for nm in PRED}
                for rt in range(NRT):
                    r0 = rt * 128
                    if USE_R["pred"] == USE_R["nce"]:
                        ccT = [(concT[rt][k2], 128) for k2 in range(i + 1)]
                    else:
                        ccT = bld.actT(concat[rt][:, :Di], Di, rdt, cpool)
                    a1o = bld.linear(ccT, pw["a1"], Di // 4, AF.Relu, ppool,
                                     name="t", tag="pa")
                    att = bld.linear(bld.actT(a1o, Di // 4, rdt, cpool),
                                     pw["a2"], Di, AF.Sigmoid, ppool,
                                     name="t", tag="pa")
                    xa = ppool.tile([128, Di], F32, name="t", tag="pa")
                    nc.vector.tensor_tensor(out=xa[:],
                                            in0=concat[rt][:, :Di],
                                            in1=att[:], op=ALU.mult)
                    if i > 0:
                        rn = bld.l2n_scale(xa, Di)
                        nc.scalar.activation(xa[:], xa[:], AF.Copy,
                                             scale=rn[:, 0:1])
                    f = bld.layernorm(bld.actT(xa, Di, rdt, cpool), pw["fe"],
                                      Hi, True, ppool)
                    for blk in ("r1", "r2"):
                        h1 = bld.layernorm(bld.actT(f, Hi, rdt, cpool),
                                           pw[blk + "a"], Mi, True, ppool)
                        h2 = bld.layernorm(bld.actT(h1, Mi, rdt, cpool),
                                           pw[blk + "b"], Hi, False, ppool)
                        f2 = ppool.tile([128, Hi], F32, name="t", tag="pa")
                        nc.vector.tensor_add(f2[:], f[:], h2[:])
                        f = f2
                    hc = bld.layernorm(bld.actT(f, Hi, rdt, cpool), pw["ca"],
                                       Mi, True, ppool)
                    tl = ppx.tile([128, ncl], F32, name="t", tag="ptl")
                    for (p, n, o) in bld.matmul_psums(
                            bld.actT(hc, Mi, rdt, cpool), pw["cb"], ncl):
                        nc.scalar.copy(tl[:, o:o + n], p[:, :n])
                    mx8 = sp.tile([128, 8], F32, name="t", tag="mx8")
                    mi8 = sp.tile([128, 8], dt.uint32, name="t", tag="mi8")
                    nc.vector.max(mx8[:], tl[:])
                    nc.vector.max_index(mi8[:], mx8[:], tl[:])
                    nmx = sp.tile([128, 1], F32, name="t", tag="nmx")
                    nc.vector.tensor_scalar_mul(nmx[:], mx8[:, 0:1], -1.0)
                    ese = sp.tile([128, 1], F32, name="t", tag="ese")
                    esc = tp("sq2", bufs=1).tile([128, 1024], F32, name="t", tag="sq2")
                    nc.scalar.activation(esc[:, :ncl], tl[:], AF.Exp,
                                         bias=nmx[:], accum_out=ese[:])
                    lse = sp.tile([128, 1], F32, name="t", tag="lse")
                    nc.scalar.activation(lse[:], ese[:], AF.Ln)
                    oht = ppx.tile([128, ncl], F32, name="t", tag="oht")
                    nc.sync.dma_start(oht[:], oh_d[i][r0:r0 + 128, :])
                    nc.vector.tensor_tensor(out=esc[:, :ncl], in0=tl[:],
                                            in1=oht[:], op=ALU.mult)
                    labl = sp.tile([128, 1], F32, name="t", tag="labl")
                    nc.vector.reduce_sum(labl[:], esc[:, :ncl], axis=AX.X)
                    pr = sp.tile([128, 1], F32, name="t", tag="pr")
                    nc.vector.tensor_tensor(out=pr[:], in0=lse[:],
                                            in1=mx8[:, 0:1], op=ALU.add)
                    nc.vector.tensor_tensor(out=pr[:], in0=pr[:],
                                            in1=labl[:], op=ALU.subtract)
                    nc.vector.tensor_add(accs["p"][:], accs["p"][:], pr[:])
                    lab = sp.tile([128, 1], dt.uint32, name="t", tag="lab")
                    nc.sync.dma_start(lab[:], lab_d[r0:r0 + 128, i:i + 1])
                    eq = sp.tile([128, 1], F32, name="t", tag="eq")
                    nc.vector.tensor_tensor(out=eq[:], in0=mi8[:, 0:1],
                                            in1=lab[:], op=ALU.is_equal)
                    nc.vector.tensor_add(accs["pc"][:], accs["pc"][:], eq[:])

        # ---- decoder ----
        with tc.tile_pool(name="decw", bufs=1) as mwp:
            dec_w = [bld.load_w(dw_d[li][:], list(dw_d[li].shape), False,
                                mwp, f"dw{li}") for li in range(4)]
            for rt in range(NRT):
                r0 = rt * 128
                rn = bld.l2n_scale(esum[rt], 128)
                xn = apool.tile([128, 128], F32, name="t", tag="h512")
                nc.scalar.activation(xn[:], esum[rt][:], AF.Copy,
                                     scale=rn[:, 0:1])
                h = bld.linear(bld.actT(xn, 128, F32, cpool), dec_w[0], 128,
                               AF.Relu, apool, name="t", tag="h512")
                h = bld.linear(bld.actT(h, 128, F32, cpool), dec_w[1], 256,
                               AF.Relu, apool, name="t", tag="h512")
                h = bld.linear(bld.actT(h, 256, F32, cpool), dec_w[2], 512,
                               AF.Relu, apool, name="t", tag="h512")
                xh = ppool.tile([128, 768], F32, name="t", tag="pa")
                for (p, n, o) in bld.matmul_psums(
                        bld.actT(h, 512, F32, cpool), dec_w[3], 768):
                    nc.scalar.copy(xh[:, o:o + n], p[:, :n])
                nc.sync.dma_start(xhat_d[r0:r0 + 128, :], xh[:])

        lac = const.tile([128, 4], F32, name="t", tag="lac")
        for j, nm in enumerate(("q", "a", "p", "pc")):
            nc.vector.tensor_copy(lac[:, j:j + 1], accs[nm][:])
        nc.sync.dma_start(lacc_d[:], lac[:])

    nc.compile()
    return nc


_NC = None


def _prep_maps(x, tags_emb, tags_indices, codebooks, params):
    jp = lambda a: np.asarray(a, np.float32)
    enc, dec = params["enc"], params["dec"]
    base = {
        "ident": np.eye(128, dtype=np.float32),
        **{f"cb{i}_": np.ascontiguousarray(codebooks[i]) for i in range(NL)},
        "cbT2": np.ascontiguousarray(
            2.0 * codebooks.transpose(0, 2, 1)).astype(np.float32),
        "cb2b": np.ascontiguousarray(np.broadcast_to(
            (codebooks.astype(np.float64) ** 2).sum(-1).astype(np.float32)
            [:, None, :], (NL, 128, K))),
    }
    for li in range(4):
        base[f"ew{li}"] = _wchunks(jp(enc[li]["w"]))
        base[f"dw{li}"] = _wchunks(jp(dec[li]["w"]))
        assert not np.any(jp(enc[li]["b"])) and not np.any(jp(dec[li]["b"]))
    for i in range(NL):
        base[f"p1_{i}"] = _wchunks(jp(params["proj"][i]["l1"]["w"]))
        base[f"p2_{i}"] = _wchunks(jp(params["proj"][i]["l2"]["w"]))
        pr = params["proj"][i]
        assert not np.any(jp(pr["l1"]["b"])) and not np.any(jp(pr["l2"]["b"]))
        assert np.all(jp(pr["bn"]["g"]) == 1) and not np.any(jp(pr["bn"]["b"]))
        pd = params["pred"][i]
        for nm in ("a1", "a2", "fe", "r1a", "r1b", "r2a", "r2b", "ca", "cb"):
            base[f"pw{i}{nm}"] = _wchunks(jp(pd[nm]["w"]))
            assert not np.any(jp(pd[nm]["b"]))
        for nm in ("fe_ln", "r1a_ln", "r1b_ln", "r2a_ln", "r2b_ln", "ca_ln"):
            assert np.all(jp(pd[nm]["g"]) == 1) and not np.any(jp(pd[nm]["b"]))

    xT = np.ascontiguousarray(x.T)
    tagsT = np.ascontiguousarray(tags_emb.transpose(1, 2, 0))
    labs = tags_indices.astype(np.uint32)
    oh = [np.eye(COUNTS[i], dtype=np.float32)[tags_indices[:, i]]
          for i in range(NL)]
    in_maps = []
    for c in range(NCORES):
        s = slice(c * BL, (c + 1) * BL)
        m = dict(base)
        m["xT"] = np.ascontiguousarray(xT[:, s])
        m["tagsT"] = np.ascontiguousarray(tagsT[:, :, s])
        m["lab"] = np.ascontiguousarray(labs[s])
        for i in range(NL):
            m[f"oh{i}"] = np.ascontiguousarray(oh[i][s])
        in_maps.append(m)
    return in_maps


def _assemble(rs):
    embs = np.concatenate([r["embs"] for r in rs], axis=1)
    resid = np.concatenate([r["resid"] for r in rs], axis=1)
    ids = np.concatenate([r["ids"] for r in rs], axis=1)
    xhat = np.concatenate([r["xhat"] for r in rs], axis=0)
    lac = np.stack([r["lacc"] for r in rs]).sum(axis=(0, 1))
    qloss = np.float32((1.0 + CW) * lac[0] / (B * D))
    aloss = np.float32(ALIGN_W * lac[1] / (B * NL))
    ploss = np.float32(lac[2] / (B * NL))
    pacc = np.float32(lac[3] / (B * NL))
    return (np.ascontiguousarray(embs.transpose(1, 2, 0)),
            np.ascontiguousarray(resid.transpose(1, 2, 0)),
            np.ascontiguousarray(ids.T).astype(np.int32),
            xhat, qloss, aloss, ploss, pacc)


def kernel(x, tags_emb, tags_indices, codebooks, params):
    global _NC
    x = np.asarray(x, np.float32)
    tags_emb = np.asarray(tags_emb, np.float32)
    tags_indices = np.asarray(tags_indices)
    codebooks = np.asarray(codebooks, np.float32)
    if _NC is None:
        _NC = build()
    in_maps = _prep_maps(x, tags_emb, tags_indices, codebooks, params)
    res = run_bass_kernel_spmd(_NC, in_maps, core_ids=list(range(NCORES)))
    return _assemble(res.results)
